# revision 26
# baseline (speedup 1.0000x reference)
"""TRN2 Bass kernel for nn_Attention_47665547051353.

Reference computation (B=4, C=512, N=2048, H=8, hd=64):
    qkv  = w_qkv @ x                           # 1x1 conv
    attn = softmax_j( k^T q * hd^-0.5 )        # softmax over QUERIES j
    out  = w_proj @ (v @ attn) + b_proj

Sharding (8 cores): core c -> batch b = c//2, head-group g = c%2 (4 heads).
Each core computes its heads' full attention plus a partial output
projection; the host sums the two partial projections per batch and adds
the bias.

Design v2 (evolved from the 219.7us baseline; see kernel_baseline.py):
  - fp16 matmuls everywhere with fp32 PSUM accumulation; inputs pre-cast
    on the host.  Softmax skips max-subtraction (scores ~N(0,1)).
  - The softmax normalizer is split across engines to balance Scalar and
    DVE (~145us each): half 0 of each unit rides the Scalar ACT
    accumulator (+187ns), half 1 is a DVE reduce_sum over the fp16 exp
    output (~1.2us).  (Notes from failed attempts: exp exists ONLY on
    the Scalar ACT; DVE tensor_scalar+accum lowers to a ~2.7us
    CACHE_REDUCE; GPSIMD can't access PSUM and its sw ops cost ~1.4us
    in launch overhead alone.)
  - Units stream head-PAIR interleaved: (t,0,i),(t,1,i),(t,0,i+1)...
    Both heads of a pair accumulate AV into one PSUM tile [128,2048]
    (head h -> partitions 64h:64h+64 via tile_position (0,64h)), so the
    8 AV matmuls per (t,i) alternate PE column groups (LDWEIGHTS always
    overlapped) and the projection contracts over clean 128-channel
    pairs -- no duplicated w_proj rows (saves 16384 PE rows).
  - Scores keep the swapped-half K/Q row-group alternation; with pair
    order the parity also alternates across unit boundaries.
  - ~10 full-tile warm-up matmuls (one accumulating group -- repeated
    closed groups with no read between wedge the PSUM bank on hw; K=64
    half-tile warmups also wedge it) ramp the PE during the DMA wait.
  - x DMAs ride the Scalar queue (Sync is blocked by the framework
    preamble until ~8us); weights are contiguous on the gpsimd queue;
    QKV matmuls chunk-accumulate as x chunks land.
  - Projection output drains per 512-chunk, copies alternating
    DVE/Scalar so the tail is PE-bound.
"""
import sys

if "/opt/trn_rl_repo" not in sys.path:
    sys.path.insert(0, "/opt/trn_rl_repo")

import numpy as np

import concourse.bass as bass
import concourse.tile as tile
import concourse.mybir as mybir
from concourse import bacc
from concourse.bass_utils import run_bass_kernel_spmd

F32 = mybir.dt.float32
F16 = mybir.dt.float16
EXP = mybir.ActivationFunctionType.Exp
MULT = mybir.AluOpType.mult

B, C, N = 4, 512, 2048
H, HD = 8, 64
SCALE = HD ** -0.5
P = 128
CC = C // P          # 4 contraction chunks over channels
NT = N // P          # 16 key blocks
HG = H // 2          # 4 heads per core (one head-group)
N_CORES = 8
N_WARM = 10          # PE p-state warm-up matmuls during input DMA

_CACHE = {}


def build_program(phases=("attn", "proj")):
    nc = bacc.Bacc("TRN2", target_bir_lowering=False, debug=False)
    x_ap = nc.dram_tensor("x", [C, N], F16, kind="ExternalInput").ap()
    # host pre-arranges to the exact SBUF layout -> contiguous DMAs
    wq_ap = nc.dram_tensor("wqT", [P, CC * HG * HD], F16, kind="ExternalInput").ap()
    wk_ap = nc.dram_tensor("wkT", [P, CC * HG * HD], F16, kind="ExternalInput").ap()
    wv_ap = nc.dram_tensor("wvT", [P, CC * HG * HD], F16, kind="ExternalInput").ap()
    wp_ap = nc.dram_tensor("wpT", [P, 2 * C], F16, kind="ExternalInput").ap()
    out_ap = nc.dram_tensor("out", [C, N], F32, kind="ExternalOutput").ap()

    with tile.TileContext(nc) as tc:
        with (
            tc.tile_pool(name="const", bufs=1) as const,
            tc.tile_pool(name="big", bufs=1) as big,
            tc.tile_pool(name="ppool", bufs=12) as ppool,
            tc.tile_pool(name="small", bufs=16) as small,
            tc.tile_pool(name="outp", bufs=4) as outp,
        ):
            # scores pool first so it owns banks 0-3; everything else
            # (warm-up scratch, QKV psum, AV accumulator) shares 4-7
            scps_cm = tc.tile_pool(name="scps", bufs=2, space="PSUM")
            scps = scps_cm.__enter__()

            QK = {}
            VT = big.tile([P, NT, HG * HD], F16)
            wp_r = const.tile([P, 2, C], F16)
            A = {}
            # pair-interleaved unit order: (t,0,i),(t,1,i),(t,0,i+1),...
            units = [(t, h, i) for t in range(2) for i in range(NT)
                     for h in range(2)]
            av_tiles = {}
            pending = {}          # (t, i) -> {h: (p_t, r_t)}
            ready = []            # [(t, i)] with both heads pending
            sc_cnt = [0]          # running score-matmul parity counter

            with tc.tile_pool(name="ld", bufs=1) as ld, \
                 tc.tile_pool(name="props", bufs=2, space="PSUM") as props:
                # ---- input DMAs ----
                # the DMA subsystem has a ~8-10us startup floor on every
                # queue (measured: moving x to the scalar queue does not
                # start it earlier), and the gpsimd queue moves weights
                # slowly (~2us per 256KB).  So the prologue-critical
                # q/k weights go FIRST on the fast sync queue, then the x
                # chunks; the late-needed v/proj weights ride gpsimd.
                wq_r = ld.tile([P, CC, HG * HD], F16)
                wk_r = ld.tile([P, CC, HG * HD], F16)
                wv_r = ld.tile([P, CC, HG * HD], F16)
                nc.sync.dma_start(out=wq_r, in_=wq_ap.rearrange("p (cc o) -> p cc o", cc=CC))
                nc.sync.dma_start(out=wk_r, in_=wk_ap.rearrange("p (cc o) -> p cc o", cc=CC))
                nc.gpsimd.dma_start(out=wv_r, in_=wv_ap.rearrange("p (cc o) -> p cc o", cc=CC))
                nc.gpsimd.dma_start(out=wp_r, in_=wp_ap.rearrange("p (t o) -> p t o", t=2))
                x_r = ld.tile([P, CC, N], F16)
                x_view = x_ap.rearrange("(cc p) n -> cc p n", p=P)
                for cc in range(CC):
                    nc.sync.dma_start(out=x_r[:, cc, :], in_=x_view[cc])

                # ACT exp-table preload + PE warm-up sources (weights and
                # moving operand must not alias in SBUF)
                warm_w = const.tile([P, P], F16, tag="warmw")
                warm_x = const.tile([P, 512], F16, tag="warmx")
                warm2 = small.tile([P, 1], F32, tag="warm2")
                warm3 = small.tile([P, 1], F32, tag="warm3")
                nc.vector.memset(warm_w, 0.0)
                nc.vector.memset(warm_x, 0.0)
                nc.vector.memset(warm2, 0.0)
                nc.scalar.activation(warm3, warm2, EXP)

                # ---- PE p-state warm-up during the DMA window ----
                # junk matmuls into one scores-pool psum buffer; WAW-chained
                # so the PE runs back-to-back and ramps to max clock
                # one ACCUMULATING group (repeated closed groups with no
                # read between them wedge the PSUM bank on real hardware),
                # closed by a tiny DVE read
                wps = scps.tile([P, 1024], F32, tag="s")
                for wi in range(N_WARM):
                    nc.tensor.matmul(
                        wps[:, 0:512], warm_w, warm_x,
                        start=(wi == 0), stop=(wi == N_WARM - 1))
                if N_WARM:
                    wrd = small.tile([P, 1], F32, tag="warm2")
                    nc.vector.tensor_copy(wrd, wps[:, 0:1])

                def emit_qk_chunk(wname, w_r, t, half):
                    """One [128,1024] output chunk of a q/k projection."""
                    key = (wname, t)
                    if key not in QK:
                        dst_new = big.tile([P, N], F16, tag=f"{wname}{t}")
                        QK[key] = dst_new
                    dst = QK[key]
                    ps = props.tile([P, 1024], F32, tag="qk")
                    for cc in range(CC):
                        for jc in range(2):
                            j0 = jc * 512
                            nc.tensor.matmul(
                                ps[:, j0:j0 + 512],
                                w_r[:, cc, t * P:(t + 1) * P],
                                x_r[:, cc, half * 1024 + j0:half * 1024 + j0 + 512],
                                start=(cc == 0), stop=(cc == CC - 1),
                            )
                    nc.vector.tensor_copy(dst[:, half * 1024:(half + 1) * 1024], ps)
                    if half == 1:
                        dstd = big.tile([P, N], F16, tag=f"{wname}d{t}")
                        nc.sync.dma_start(out=dstd[0:HD, :], in_=dst[HD:2 * HD, :])
                        nc.sync.dma_start(out=dstd[HD:2 * HD, :], in_=dst[0:HD, :])
                        QK[(wname + "d", t)] = dstd

                def emit_v_chunk(vt2, half, vr):
                    ps = props.tile([P, 1024], F32, tag="qk")
                    for cc in range(CC):
                        for jc in range(2):
                            j0 = jc * 512
                            nc.tensor.matmul(
                                ps[:, j0:j0 + 512],
                                wv_r[:, cc, vt2 * P:(vt2 + 1) * P],
                                x_r[:, cc, half * 1024 + j0:half * 1024 + j0 + 512],
                                start=(cc == 0), stop=(cc == CC - 1),
                            )
                    nc.vector.tensor_copy(vr[:, half * 1024:(half + 1) * 1024], ps)
                    if half == 1:
                        for nt in range(NT):
                            nc.sync.dma_start(
                                out=VT[:, nt, vt2 * P:(vt2 + 1) * P],
                                in_=vr[:, nt * P:(nt + 1) * P],
                                transpose=True,
                            )

                def emit_unit(t, h, i, first_units=False):
                    """Scores + exp + normalizer for one (head, key-block).

                    The normalizer sum is split across engines to balance
                    them: half 0 rides the Scalar ACT accumulator (+187ns
                    Scalar), half 1 is a DVE reduce over the fp16 output
                    (~1.2us DVE)."""
                    p_t = ppool.tile([P, N], F16, tag="p")
                    sv0 = small.tile([P, 1], F32, tag="sv0")
                    for half in range(2):
                        sps = scps.tile([P, 1024], F32, tag="s")
                        for jc in range(2):
                            # choose the physical PE row group by a global
                            # running parity so consecutive score matmuls
                            # always alternate row groups (LDW overlap);
                            # the swapped-half K/Q copies make either row
                            # group serve either head
                            if first_units:
                                # swapped copies not DMA'd yet
                                kk, qq, rb = QK[("k", t)], QK[("q", t)], h * HD
                                sc_cnt[0] = h + 1
                            else:
                                par = sc_cnt[0] % 2
                                sc_cnt[0] += 1
                                if par == h:
                                    kk, qq, rb = QK[("k", t)], QK[("q", t)], h * HD
                                else:
                                    kk, qq, rb = QK[("kd", t)], QK[("qd", t)], par * HD
                            nc.tensor.matmul(
                                sps[:, jc * 512:(jc + 1) * 512],
                                kk[rb:rb + HD, i * P:(i + 1) * P],
                                qq[rb:rb + HD,
                                   half * 1024 + jc * 512:half * 1024 + (jc + 1) * 512],
                                start=True, stop=True,
                            )
                        nc.scalar.activation(
                            p_t[:, half * 1024:(half + 1) * 1024], sps,
                            EXP, scale=SCALE,
                            accum_out=sv0 if half == 0 else None)
                    sv1 = small.tile([P, 1], F32, tag="sv1")
                    nc.vector.reduce_sum(sv1, p_t[:, 1024:2048],
                                         axis=mybir.AxisListType.X)
                    s_all = small.tile([P, 1], F32, tag="stot")
                    nc.vector.tensor_add(s_all, sv0, sv1)
                    r_t = small.tile([P, 1], F32, tag="rcp")
                    nc.vector.reciprocal(r_t, s_all)
                    return p_t, r_t

                def emit_av_pair(t, i, u0, u1):
                    """8 AV matmuls for both heads of pair t at key-block i,
                    alternating PE column groups."""
                    vps = []
                    for h, (p_t, r_t) in ((0, u0), (1, u1)):
                        hl = 2 * t + h
                        vp = small.tile([P, HD], F16, tag=f"vp{h}")
                        nc.vector.tensor_scalar_mul(
                            vp, VT[:, i, hl * HD:(hl + 1) * HD], r_t)
                        vps.append(vp)
                    if t not in av_tiles:
                        av_tiles[t] = avps.tile([P, N], F32, tag="av",
                                                name=f"av{t}")
                    av = av_tiles[t]
                    for jc4 in range(4):
                        for h, (p_t, r_t) in ((0, u0), (1, u1)):
                            nc.tensor.matmul(
                                av[h * HD:(h + 1) * HD,
                                   jc4 * 512:(jc4 + 1) * 512],
                                vps[h],
                                p_t[:, jc4 * 512:(jc4 + 1) * 512],
                                start=(i == 0), stop=(i == NT - 1),
                                tile_position=(0, h * HD),
                                skip_group_check=True,
                            )
                        if i == NT - 1:
                            # column jc4 complete for both heads: cast out.
                            # For the last pair (tail) alternate onto the
                            # now-idle Scalar engine.
                            a_t = A.get(t)
                            if a_t is None:
                                a_t = big.tile([P, N], F16, tag=f"a{t}")
                                A[t] = a_t
                            if t == 1 and jc4 % 2 == 0:
                                nc.scalar.copy(
                                    a_t[:, jc4 * 512:(jc4 + 1) * 512],
                                    av[:, jc4 * 512:(jc4 + 1) * 512])
                            else:
                                nc.vector.tensor_copy(
                                    a_t[:, jc4 * 512:(jc4 + 1) * 512],
                                    av[:, jc4 * 512:(jc4 + 1) * 512])
                    if i == NT - 1:
                        av_tiles.pop(t)

                def note_unit(u, res):
                    t, h, i = u
                    pending.setdefault((t, i), {})[h] = res
                    if len(pending[(t, i)]) == 2:
                        ready.append((t, i))

                def drain(depth):
                    while len(ready) > depth:
                        (t, i) = ready.pop(0)
                        d = pending.pop((t, i))
                        emit_av_pair(t, i, d[0], d[1])

                # ---- prologue: q0 + k0, then first units + fill ----
                emit_qk_chunk("q", wq_r, 0, 0)
                emit_qk_chunk("q", wq_r, 0, 1)
                emit_qk_chunk("k", wk_r, 0, 0)

                vrow0 = ld.tile([P, N], F16, tag="vrow0")
                vrow1 = ld.tile([P, N], F16, tag="vrow1")
                vrow = [vrow0, vrow1]
                fill = [lambda: emit_qk_chunk("k", wk_r, 0, 1),
                        lambda: emit_v_chunk(0, 0, vrow[0]),
                        lambda: emit_v_chunk(0, 1, vrow[0]),
                        lambda: emit_v_chunk(1, 0, vrow[1]),
                        lambda: emit_v_chunk(1, 1, vrow[1]),
                        lambda: emit_qk_chunk("q", wq_r, 1, 0),
                        lambda: emit_qk_chunk("q", wq_r, 1, 1),
                        lambda: emit_qk_chunk("k", wk_r, 1, 0),
                        lambda: emit_qk_chunk("k", wk_r, 1, 1)]
                n_pre = min(8, len(units)) if ("attn" in phases) else 0
                for g in range(n_pre):
                    u = units[g]
                    note_unit(u, emit_unit(*u, first_units=True))
                    if g < len(fill):
                        fill[g]()
                for f in fill[n_pre:]:
                    f()

            # ---- main attention stream ----
            with tc.tile_pool(name="avps", bufs=1, space="PSUM") as avps:
              if "attn" in phases:
                for g in range(n_pre, len(units)):
                    u = units[g]
                    note_unit(u, emit_unit(*u))
                    # lead shrinks 4 pairs -> 1 as the stream winds down
                    gl = len(units) - 1 - g
                    drain(min(4, max(1, gl // 2)))
                drain(0)

            scps_cm.__exit__(None, None, None)

            # ---- output projection (clean 128-channel pair chunks) ----
            with tc.tile_pool(name="prps", bufs=2, space="PSUM") as prps:
              if "proj" in phases and len(A) == 2:
                for ot in range(4):
                    pso = prps.tile([P, N], F32)
                    for jc in range(4):
                        for t2 in range(2):
                            nc.tensor.matmul(
                                pso[:, jc * 512:(jc + 1) * 512],
                                wp_r[:, t2, ot * P:(ot + 1) * P],
                                A[t2][:, jc * 512:(jc + 1) * 512],
                                start=(t2 == 0), stop=(t2 == 1),
                            )
                        # copy each 512-chunk out as soon as its stop
                        # matmul lands; alternate DVE/Scalar (Scalar is
                        # idle in the tail) so copies never gate the PE
                        o_sb = outp.tile([P, 512], F32, tag="o")
                        if jc % 2 == 0:
                            nc.scalar.copy(
                                o_sb, pso[:, jc * 512:(jc + 1) * 512])
                        else:
                            nc.vector.tensor_copy(
                                o_sb, pso[:, jc * 512:(jc + 1) * 512])
                        nc.sync.dma_start(
                            out=out_ap[ot * P:(ot + 1) * P,
                                       jc * 512:(jc + 1) * 512],
                            in_=o_sb)

    nc.compile()
    return nc


def _shard_weights(w_qkv, w_proj):
    """Per head-group g: q/k/v weight shards pre-arranged to the SBUF
    layout [P, CC, 256] (row p of contraction chunk cc; output column
    o = 64*h_local + d), and projection shard [P, 2, C] (pair-chunk t
    rows 64*h_in_pair + d)."""
    shards = []
    for g in range(2):
        heads = range(HG * g, HG * (g + 1))
        q_rows = [h * 3 * HD + d for h in heads for d in range(HD)]
        k_rows = [h * 3 * HD + HD + d for h in heads for d in range(HD)]
        v_rows = [h * 3 * HD + 2 * HD + d for h in heads for d in range(HD)]

        def arrange_qkv(rows):
            w = w_qkv[rows, :].T                       # [C, 256]
            w = w.reshape(CC, P, HG * HD)              # [cc, p, o]
            return np.ascontiguousarray(
                w.transpose(1, 0, 2).reshape(P, CC * HG * HD))

        # wp chunk t rows: channel of head (4g + 2t + h_in_pair), dim d
        wp = np.empty((P, 2, C), dtype=w_proj.dtype)
        for t in range(2):
            chans = [(4 * g + 2 * t + hh) * HD + d
                     for hh in range(2) for d in range(HD)]
            wp[:, t, :] = w_proj[:, chans].T
        shards.append({
            "wqT": arrange_qkv(q_rows),
            "wkT": arrange_qkv(k_rows),
            "wvT": arrange_qkv(v_rows),
            "wpT": np.ascontiguousarray(wp.reshape(P, 2 * C)),
        })
    return shards


def kernel(x, w_qkv, w_proj, b_proj, _trace=False, _trace_kwargs=None):
    x = np.asarray(x, dtype=np.float32)
    w_qkv = np.asarray(w_qkv, dtype=np.float32)
    w_proj = np.asarray(w_proj, dtype=np.float32)
    b_proj = np.asarray(b_proj, dtype=np.float32)

    if "nc" not in _CACHE:
        _CACHE["nc"] = build_program()
    nc = _CACHE["nc"]

    shards = _shard_weights(w_qkv, w_proj)
    shards = [{k: v.astype(np.float16) for k, v in s.items()} for s in shards]
    in_maps = []
    for core in range(N_CORES):
        b, g = core // 2, core % 2
        m = {"x": np.ascontiguousarray(x[b].astype(np.float16))}
        m.update(shards[g])
        in_maps.append(m)

    kw = {}
    if _trace:
        kw.update(trace=True, trace_cores=[0], **(_trace_kwargs or {}))
    res = run_bass_kernel_spmd(nc, in_maps, list(range(N_CORES)), **kw)

    out = np.empty((B, C, N), dtype=np.float32)
    for b in range(B):
        out[b] = (res.results[2 * b]["out"] + res.results[2 * b + 1]["out"]
                  + b_proj[:, None])
    if _trace:
        _CACHE["last_result"] = res
    return out


# revision 29
# speedup vs baseline: 1.0537x; 1.0537x over previous
"""TRN2 Bass kernel for nn_Attention_47665547051353.

Reference computation (B=4, C=512, N=2048, H=8, hd=64):
    qkv  = w_qkv @ x                           # 1x1 conv
    attn = softmax_j( k^T q * hd^-0.5 )        # softmax over QUERIES j
    out  = w_proj @ (v @ attn) + b_proj

Sharding (8 cores): core c -> batch b = c//2, head-group g = c%2 (4 heads).
Each core computes its heads' full attention plus a partial output
projection; the host sums the two partial projections per batch and adds
the bias.

Design v2 (evolved from the 219.7us baseline; see kernel_baseline.py):
  - fp16 matmuls everywhere with fp32 PSUM accumulation; inputs pre-cast
    on the host.  Softmax skips max-subtraction (scores ~N(0,1)).
  - The softmax normalizer is split across engines to balance Scalar and
    DVE (~145us each): half 0 of each unit rides the Scalar ACT
    accumulator (+187ns), half 1 is a DVE reduce_sum over the fp16 exp
    output (~1.2us).  (Notes from failed attempts: exp exists ONLY on
    the Scalar ACT; DVE tensor_scalar+accum lowers to a ~2.7us
    CACHE_REDUCE; GPSIMD can't access PSUM and its sw ops cost ~1.4us
    in launch overhead alone.)
  - Units stream head-PAIR interleaved: (t,0,i),(t,1,i),(t,0,i+1)...
    Both heads of a pair accumulate AV into one PSUM tile [128,2048]
    (head h -> partitions 64h:64h+64 via tile_position (0,64h)), so the
    8 AV matmuls per (t,i) alternate PE column groups (LDWEIGHTS always
    overlapped) and the projection contracts over clean 128-channel
    pairs -- no duplicated w_proj rows (saves 16384 PE rows).
  - Scores keep the swapped-half K/Q row-group alternation; with pair
    order the parity also alternates across unit boundaries.
  - ~10 full-tile warm-up matmuls (one accumulating group -- repeated
    closed groups with no read between wedge the PSUM bank on hw; K=64
    half-tile warmups also wedge it) ramp the PE during the DMA wait.
  - x DMAs ride the Scalar queue (Sync is blocked by the framework
    preamble until ~8us); weights are contiguous on the gpsimd queue;
    QKV matmuls chunk-accumulate as x chunks land.
  - Projection output drains per 512-chunk, copies alternating
    DVE/Scalar so the tail is PE-bound.
"""
import sys

if "/opt/trn_rl_repo" not in sys.path:
    sys.path.insert(0, "/opt/trn_rl_repo")

import numpy as np

import concourse.bass as bass
import concourse.tile as tile
import concourse.mybir as mybir
from concourse import bacc
from concourse.bass_utils import run_bass_kernel_spmd

F32 = mybir.dt.float32
F16 = mybir.dt.float16
EXP = mybir.ActivationFunctionType.Exp
MULT = mybir.AluOpType.mult

B, C, N = 4, 512, 2048
H, HD = 8, 64
SCALE = HD ** -0.5
P = 128
CC = C // P          # 4 contraction chunks over channels
NT = N // P          # 16 key blocks
HG = H // 2          # 4 heads per core (one head-group)
N_CORES = 8
N_WARM = 20          # PE p-state warm-up matmuls during input DMA

_CACHE = {}


def build_program(phases=("attn", "proj")):
    nc = bacc.Bacc("TRN2", target_bir_lowering=False, debug=False)
    x_ap = nc.dram_tensor("x", [C, N], F16, kind="ExternalInput").ap()
    # host pre-arranges to the exact SBUF layout -> contiguous DMAs
    wq_ap = nc.dram_tensor("wqT", [P, CC * HG * HD], F16, kind="ExternalInput").ap()
    wk_ap = nc.dram_tensor("wkT", [P, CC * HG * HD], F16, kind="ExternalInput").ap()
    wv_ap = nc.dram_tensor("wvT", [P, CC * HG * HD], F16, kind="ExternalInput").ap()
    wp_ap = nc.dram_tensor("wpT", [P, 2 * C], F16, kind="ExternalInput").ap()
    out_ap = nc.dram_tensor("out", [C, N], F32, kind="ExternalOutput").ap()

    with tile.TileContext(nc) as tc:
        with (
            tc.tile_pool(name="const", bufs=1) as const,
            tc.tile_pool(name="big", bufs=1) as big,
            tc.tile_pool(name="ppool", bufs=12) as ppool,
            tc.tile_pool(name="small", bufs=16) as small,
            tc.tile_pool(name="outp", bufs=4) as outp,
        ):
            # scores pool first so it owns banks 0-3; everything else
            # (warm-up scratch, QKV psum, AV accumulator) shares 4-7
            scps_cm = tc.tile_pool(name="scps", bufs=2, space="PSUM")
            scps = scps_cm.__enter__()

            QK = {}
            VT = big.tile([P, NT, HG * HD], F16)
            wp_r = const.tile([P, 2, C], F16)
            A = {}
            # pair-interleaved unit order: (t,0,i),(t,1,i),(t,0,i+1),...
            units = [(t, h, i) for t in range(2) for i in range(NT)
                     for h in range(2)]
            av_tiles = {}
            pending = {}          # (t, i) -> {h: (p_t, r_t)}
            ready = []            # [(t, i)] with both heads pending
            sc_cnt = [0]          # running score-matmul parity counter

            with tc.tile_pool(name="ld", bufs=1) as ld, \
                 tc.tile_pool(name="props", bufs=2, space="PSUM") as props:
                # ---- input DMAs ----
                # the DMA subsystem has a ~8-10us startup floor on every
                # queue (measured: moving x to the scalar queue does not
                # start it earlier), and the gpsimd queue moves weights
                # slowly (~2us per 256KB).  So the prologue-critical
                # q/k weights go FIRST on the fast sync queue, then the x
                # chunks; the late-needed v/proj weights ride gpsimd.
                wq_r = ld.tile([P, CC, HG * HD], F16)
                wk_r = ld.tile([P, CC, HG * HD], F16)
                wv_r = ld.tile([P, CC, HG * HD], F16)
                nc.sync.dma_start(out=wq_r, in_=wq_ap.rearrange("p (cc o) -> p cc o", cc=CC))
                nc.sync.dma_start(out=wk_r, in_=wk_ap.rearrange("p (cc o) -> p cc o", cc=CC))
                nc.gpsimd.dma_start(out=wv_r, in_=wv_ap.rearrange("p (cc o) -> p cc o", cc=CC))
                nc.gpsimd.dma_start(out=wp_r, in_=wp_ap.rearrange("p (t o) -> p t o", t=2))
                x_r = ld.tile([P, CC, N], F16)
                x_view = x_ap.rearrange("(cc p) n -> cc p n", p=P)
                for cc in range(CC):
                    nc.sync.dma_start(out=x_r[:, cc, :], in_=x_view[cc])

                # ACT exp-table preload + PE warm-up sources (weights and
                # moving operand must not alias in SBUF)
                warm_w = const.tile([P, P], F16, tag="warmw")
                warm_x = const.tile([P, 512], F16, tag="warmx")
                warm2 = small.tile([P, 1], F32, tag="warm2")
                warm3 = small.tile([P, 1], F32, tag="warm3")
                nc.vector.memset(warm_w, 0.0)
                nc.vector.memset(warm_x, 0.0)
                nc.vector.memset(warm2, 0.0)
                nc.scalar.activation(warm3, warm2, EXP)

                # ---- PE p-state warm-up during the DMA window ----
                # junk matmuls into one scores-pool psum buffer; WAW-chained
                # so the PE runs back-to-back and ramps to max clock
                # one ACCUMULATING group (repeated closed groups with no
                # read between them wedge the PSUM bank on real hardware),
                # closed by a tiny DVE read
                wps = scps.tile([P, 1024], F32, tag="s")
                for wi in range(N_WARM):
                    nc.tensor.matmul(
                        wps[:, 0:512], warm_w, warm_x,
                        start=(wi == 0), stop=(wi == N_WARM - 1))
                if N_WARM:
                    wrd = small.tile([P, 1], F32, tag="warm2")
                    nc.vector.tensor_copy(wrd, wps[:, 0:1])

                def emit_qk_chunk(wname, w_r, t, half):
                    """One [128,1024] output chunk of a q/k projection."""
                    key = (wname, t)
                    if key not in QK:
                        dst_new = big.tile([P, N], F16, tag=f"{wname}{t}")
                        QK[key] = dst_new
                    dst = QK[key]
                    ps = props.tile([P, 1024], F32, tag="qk")
                    for cc in range(CC):
                        for jc in range(2):
                            j0 = jc * 512
                            nc.tensor.matmul(
                                ps[:, j0:j0 + 512],
                                w_r[:, cc, t * P:(t + 1) * P],
                                x_r[:, cc, half * 1024 + j0:half * 1024 + j0 + 512],
                                start=(cc == 0), stop=(cc == CC - 1),
                            )
                    nc.vector.tensor_copy(dst[:, half * 1024:(half + 1) * 1024], ps)
                    if half == 1:
                        dstd = big.tile([P, N], F16, tag=f"{wname}d{t}")
                        nc.sync.dma_start(out=dstd[0:HD, :], in_=dst[HD:2 * HD, :])
                        nc.sync.dma_start(out=dstd[HD:2 * HD, :], in_=dst[0:HD, :])
                        QK[(wname + "d", t)] = dstd

                def emit_v_chunk(vt2, half, vr):
                    ps = props.tile([P, 1024], F32, tag="qk")
                    for cc in range(CC):
                        for jc in range(2):
                            j0 = jc * 512
                            nc.tensor.matmul(
                                ps[:, j0:j0 + 512],
                                wv_r[:, cc, vt2 * P:(vt2 + 1) * P],
                                x_r[:, cc, half * 1024 + j0:half * 1024 + j0 + 512],
                                start=(cc == 0), stop=(cc == CC - 1),
                            )
                    nc.vector.tensor_copy(vr[:, half * 1024:(half + 1) * 1024], ps)
                    if half == 1:
                        for nt in range(NT):
                            nc.sync.dma_start(
                                out=VT[:, nt, vt2 * P:(vt2 + 1) * P],
                                in_=vr[:, nt * P:(nt + 1) * P],
                                transpose=True,
                            )

                def emit_unit(t, h, i, first_units=False):
                    """Scores + exp + normalizer for one (head, key-block).

                    The normalizer sum is split across engines to balance
                    them: half 0 rides the Scalar ACT accumulator (+187ns
                    Scalar), half 1 is a DVE reduce over the fp16 output
                    (~1.2us DVE)."""
                    p_t = ppool.tile([P, N], F16, tag="p")
                    sv0 = small.tile([P, 1], F32, tag="sv0")
                    for half in range(2):
                        sps = scps.tile([P, 1024], F32, tag="s")
                        for jc in range(2):
                            # choose the physical PE row group by a global
                            # running parity so consecutive score matmuls
                            # always alternate row groups (LDW overlap);
                            # the swapped-half K/Q copies make either row
                            # group serve either head
                            if first_units:
                                # swapped copies not DMA'd yet
                                kk, qq, rb = QK[("k", t)], QK[("q", t)], h * HD
                                sc_cnt[0] = h + 1
                            else:
                                par = sc_cnt[0] % 2
                                sc_cnt[0] += 1
                                if par == h:
                                    kk, qq, rb = QK[("k", t)], QK[("q", t)], h * HD
                                else:
                                    kk, qq, rb = QK[("kd", t)], QK[("qd", t)], par * HD
                            nc.tensor.matmul(
                                sps[:, jc * 512:(jc + 1) * 512],
                                kk[rb:rb + HD, i * P:(i + 1) * P],
                                qq[rb:rb + HD,
                                   half * 1024 + jc * 512:half * 1024 + (jc + 1) * 512],
                                start=True, stop=True,
                            )
                        nc.scalar.activation(
                            p_t[:, half * 1024:(half + 1) * 1024], sps,
                            EXP, scale=SCALE,
                            accum_out=sv0 if half == 0 else None)
                    sv1 = small.tile([P, 1], F32, tag="sv1")
                    nc.vector.reduce_sum(sv1, p_t[:, 1024:2048],
                                         axis=mybir.AxisListType.X)
                    s_all = small.tile([P, 1], F32, tag="stot")
                    nc.vector.tensor_add(s_all, sv0, sv1)
                    r_t = small.tile([P, 1], F32, tag="rcp")
                    nc.vector.reciprocal(r_t, s_all)
                    return p_t, r_t

                def emit_av_pair(t, i, u0, u1):
                    """8 AV matmuls for both heads of pair t at key-block i,
                    alternating PE column groups."""
                    vps = []
                    for h, (p_t, r_t) in ((0, u0), (1, u1)):
                        hl = 2 * t + h
                        vp = small.tile([P, HD], F16, tag=f"vp{h}")
                        nc.vector.tensor_scalar_mul(
                            vp, VT[:, i, hl * HD:(hl + 1) * HD], r_t)
                        vps.append(vp)
                    if t not in av_tiles:
                        av_tiles[t] = avps.tile([P, N], F32, tag="av",
                                                name=f"av{t}")
                    av = av_tiles[t]
                    for jc4 in range(4):
                        for h, (p_t, r_t) in ((0, u0), (1, u1)):
                            nc.tensor.matmul(
                                av[h * HD:(h + 1) * HD,
                                   jc4 * 512:(jc4 + 1) * 512],
                                vps[h],
                                p_t[:, jc4 * 512:(jc4 + 1) * 512],
                                start=(i == 0), stop=(i == NT - 1),
                                tile_position=(0, h * HD),
                                skip_group_check=True,
                            )
                        if i == NT - 1:
                            # column jc4 complete for both heads: cast out
                            # (all on DVE -- pushing casts/copies onto the
                            # Scalar engine in the tail stalls the PE and
                            # collapses its p-state, measured +10us)
                            a_t = A.get(t)
                            if a_t is None:
                                a_t = big.tile([P, N], F16, tag=f"a{t}")
                                A[t] = a_t
                            nc.vector.tensor_copy(
                                a_t[:, jc4 * 512:(jc4 + 1) * 512],
                                av[:, jc4 * 512:(jc4 + 1) * 512])
                    if i == NT - 1:
                        av_tiles.pop(t)

                def note_unit(u, res):
                    t, h, i = u
                    pending.setdefault((t, i), {})[h] = res
                    if len(pending[(t, i)]) == 2:
                        ready.append((t, i))

                def drain(depth):
                    while len(ready) > depth:
                        (t, i) = ready.pop(0)
                        d = pending.pop((t, i))
                        emit_av_pair(t, i, d[0], d[1])

                # ---- prologue: q0 + k0, then first units + fill ----
                emit_qk_chunk("q", wq_r, 0, 0)
                emit_qk_chunk("q", wq_r, 0, 1)
                emit_qk_chunk("k", wk_r, 0, 0)

                vrow0 = ld.tile([P, N], F16, tag="vrow0")
                vrow1 = ld.tile([P, N], F16, tag="vrow1")
                vrow = [vrow0, vrow1]
                fill = [lambda: emit_qk_chunk("k", wk_r, 0, 1),
                        lambda: emit_v_chunk(0, 0, vrow[0]),
                        lambda: emit_v_chunk(0, 1, vrow[0]),
                        lambda: emit_v_chunk(1, 0, vrow[1]),
                        lambda: emit_v_chunk(1, 1, vrow[1]),
                        lambda: emit_qk_chunk("q", wq_r, 1, 0),
                        lambda: emit_qk_chunk("q", wq_r, 1, 1),
                        lambda: emit_qk_chunk("k", wk_r, 1, 0),
                        lambda: emit_qk_chunk("k", wk_r, 1, 1)]
                n_pre = min(8, len(units)) if ("attn" in phases) else 0
                for g in range(n_pre):
                    u = units[g]
                    note_unit(u, emit_unit(*u, first_units=True))
                    if g < len(fill):
                        fill[g]()
                for f in fill[n_pre:]:
                    f()

            # ---- main attention stream ----
            with tc.tile_pool(name="avps", bufs=1, space="PSUM") as avps:
              if "attn" in phases:
                for g in range(n_pre, len(units)):
                    u = units[g]
                    note_unit(u, emit_unit(*u))
                    # lead shrinks 4 pairs -> 1 as the stream winds down
                    gl = len(units) - 1 - g
                    drain(min(4, max(1, gl // 2)))
                drain(0)

            scps_cm.__exit__(None, None, None)

            # ---- output projection (clean 128-channel pair chunks) ----
            with tc.tile_pool(name="prps", bufs=2, space="PSUM") as prps:
              if "proj" in phases and len(A) == 2:
                for ot in range(4):
                    pso = prps.tile([P, N], F32)
                    for jc in range(4):
                        for t2 in range(2):
                            nc.tensor.matmul(
                                pso[:, jc * 512:(jc + 1) * 512],
                                wp_r[:, t2, ot * P:(ot + 1) * P],
                                A[t2][:, jc * 512:(jc + 1) * 512],
                                start=(t2 == 0), stop=(t2 == 1),
                            )
                    for oc in range(2):
                        o_sb = outp.tile([P, 1024], F32, tag="o")
                        nc.vector.tensor_copy(
                            o_sb, pso[:, oc * 1024:(oc + 1) * 1024])
                        nc.sync.dma_start(
                            out=out_ap[ot * P:(ot + 1) * P,
                                       oc * 1024:(oc + 1) * 1024],
                            in_=o_sb)

    nc.compile()
    return nc


def _shard_weights(w_qkv, w_proj):
    """Per head-group g: q/k/v weight shards pre-arranged to the SBUF
    layout [P, CC, 256] (row p of contraction chunk cc; output column
    o = 64*h_local + d), and projection shard [P, 2, C] (pair-chunk t
    rows 64*h_in_pair + d)."""
    shards = []
    for g in range(2):
        heads = range(HG * g, HG * (g + 1))
        q_rows = [h * 3 * HD + d for h in heads for d in range(HD)]
        k_rows = [h * 3 * HD + HD + d for h in heads for d in range(HD)]
        v_rows = [h * 3 * HD + 2 * HD + d for h in heads for d in range(HD)]

        def arrange_qkv(rows):
            w = w_qkv[rows, :].T                       # [C, 256]
            w = w.reshape(CC, P, HG * HD)              # [cc, p, o]
            return np.ascontiguousarray(
                w.transpose(1, 0, 2).reshape(P, CC * HG * HD))

        # wp chunk t rows: channel of head (4g + 2t + h_in_pair), dim d
        wp = np.empty((P, 2, C), dtype=w_proj.dtype)
        for t in range(2):
            chans = [(4 * g + 2 * t + hh) * HD + d
                     for hh in range(2) for d in range(HD)]
            wp[:, t, :] = w_proj[:, chans].T
        shards.append({
            "wqT": arrange_qkv(q_rows),
            "wkT": arrange_qkv(k_rows),
            "wvT": arrange_qkv(v_rows),
            "wpT": np.ascontiguousarray(wp.reshape(P, 2 * C)),
        })
    return shards


def kernel(x, w_qkv, w_proj, b_proj, _trace=False, _trace_kwargs=None):
    x = np.asarray(x, dtype=np.float32)
    w_qkv = np.asarray(w_qkv, dtype=np.float32)
    w_proj = np.asarray(w_proj, dtype=np.float32)
    b_proj = np.asarray(b_proj, dtype=np.float32)

    if "nc" not in _CACHE:
        _CACHE["nc"] = build_program()
    nc = _CACHE["nc"]

    shards = _shard_weights(w_qkv, w_proj)
    shards = [{k: v.astype(np.float16) for k, v in s.items()} for s in shards]
    in_maps = []
    for core in range(N_CORES):
        b, g = core // 2, core % 2
        m = {"x": np.ascontiguousarray(x[b].astype(np.float16))}
        m.update(shards[g])
        in_maps.append(m)

    kw = {}
    if _trace:
        kw.update(trace=True, trace_cores=[0], **(_trace_kwargs or {}))
    res = run_bass_kernel_spmd(nc, in_maps, list(range(N_CORES)), **kw)

    out = np.empty((B, C, N), dtype=np.float32)
    for b in range(B):
        out[b] = (res.results[2 * b]["out"] + res.results[2 * b + 1]["out"]
                  + b_proj[:, None])
    if _trace:
        _CACHE["last_result"] = res
    return out
